# revision 2
# baseline (speedup 1.0000x reference)
"""Trainium2 Bass kernel for nn_AttentionModel (4-layer gated transformer).

Sharding: pure data-parallel over batch. B=16 -> 2 batch elements per core
across 8 NeuronCores; no collectives. Activations live feature-major
([feature_partition, token_free]) so every projection is a natural PE matmul
with the weight as the stationary operand. LayerNorm partition-reductions and
partition-broadcasts go through the PE via ones-vector matmuls. Softmax runs
in scores-transposed layout (keys on partitions): denominators come from a
ones-augmented column of V during the context matmul, so no probs transpose
is needed. Matmuls run in float32r (full-rate reduced-precision fp32).
"""

import os
import sys

for _p in ("/opt/trn_rl_repo",):
    if os.path.isdir(_p) and _p not in sys.path:
        sys.path.insert(0, _p)

import numpy as np

import concourse.bass as bass
import concourse.mybir as mybir
import concourse.tile as tile
from concourse import bacc
from concourse.bass_utils import run_bass_kernel_spmd

F32 = mybir.dt.float32
F32R = mybir.dt.float32r
AF = mybir.ActivationFunctionType
OP = mybir.AluOpType

B, S, FC, FO = 16, 512, 24, 16
D, H, DK, FFD, L = 512, 8, 64, 2048, 4
MAXPOS = 512
EPS = 1e-6

NCORES = 8
BL = B // NCORES          # local batch = 2
R = BL * S                # local tokens = 1024
RC = R // 512             # 512-wide token chunks = 2
DT = D // 128             # feature tiles = 4
FT = FFD // 128           # ff tiles = 16
HDK = H * DK

_CACHE = {}


def _build():
    nc = bacc.Bacc("TRN2", target_bir_lowering=False, debug=False,
                   num_devices=NCORES)

    def par(name, shape, dt=F32R):
        return nc.declare_dram_parameter(name, list(shape), dt, isOutput=False)

    x_cgmT = par("x_cgmT", [FC, R])
    x_otherT = par("x_otherT", [FO, BL])
    cgm_W = par("cgm_W", [FC, D])
    cgm_b = par("cgm_b", [D], F32)
    rel_embT = par("rel_embT", [DK, 2 * MAXPOS - 1], F32)
    Jx = par("J", [128, 128], F32)
    Wq = par("Wq", [L, D, HDK]); bq = par("bq", [L, HDK], F32)
    Wk = par("Wk", [L, D, HDK]); bk = par("bk", [L, HDK], F32)
    Wv = par("Wv", [L, D, HDK]); bv = par("bv", [L, HDK], F32)
    Wo = par("Wo", [L, HDK, D]); bo = par("bo", [L, D], F32)
    Wg = par("Wg", [L, D, D]);   bg = par("bg", [L, D], F32)
    Wf1 = par("Wf1", [L, D, FFD]); bf1 = par("bf1", [L, FFD], F32)
    Wfg = par("Wfg", [L, D, FFD]); bfg = par("bfg", [L, FFD], F32)
    Wf2 = par("Wf2", [L, FFD, D]); bf2 = par("bf2", [L, D], F32)
    ln1_s = par("ln1_s", [L, D], F32); ln1_b = par("ln1_b", [L, D], F32)
    ln2_s = par("ln2_s", [L, D], F32); ln2_b = par("ln2_b", [L, D], F32)
    other_W = par("other_W", [FO, D])
    other_b = par("other_b", [D], F32)
    fW1 = par("fW1", [2 * D, 256]); fb1 = par("fb1", [256], F32)
    fln1_s = par("fln1_s", [256], F32); fln1_b = par("fln1_b", [256], F32)
    fW2 = par("fW2", [256, 128]); fb2 = par("fb2", [128], F32)
    fln2_s = par("fln2_s", [128], F32); fln2_b = par("fln2_b", [128], F32)
    fW3 = par("fW3", [128, 1]); fb3 = par("fb3", [1], F32)
    out_ext = nc.declare_dram_parameter("out", [1, BL], F32, isOutput=True)

    rbar_dram = nc.dram_tensor("rbar", [2 * MAXPOS - 1], F32)

    with tile.TileContext(nc) as tc:
        with (
            nc.allow_low_precision(reason="float32r matmul operand rounding"),
            tc.tile_pool(name="P", bufs=1) as P,
            tc.tile_pool(name="Q", bufs=1, space="PSUM") as Q,
        ):
          try:
            MM = nc.tensor.matmul
            KSTOP = os.environ.get("KSTOP", "full")

            def early_out():
                zo = P.tile([1, BL], F32, tag="outsb", bufs=1)
                nc.vector.memset(zo, 0.0)
                nc.sync.dma_start(out=out_ext[:, :], in_=zo)

            class _Stop(Exception):
                pass

            def gate(stage):
                if KSTOP == stage:
                    early_out()
                    raise _Stop()

            # ---------------- constants ----------------
            def const_tile(shape, value, tag):
                f = P.tile(shape, F32, tag=tag + "f", bufs=1)
                nc.vector.memset(f, value)
                r_ = P.tile(shape, F32R, tag=tag, bufs=1)
                nc.vector.tensor_copy(r_, f)
                return r_

            ones_row = const_tile([1, 128], 1.0, "c_or")   # bcast lhsT
            ones_col = const_tile([128, 1], 1.0, "c_oc")   # LN-sum lhsT
            ones8 = const_tile([128, 8], 1.0, "c_o8")      # v ones columns
            o64m = P.tile([DK, 1], F32, tag="c_m", bufs=1)
            nc.vector.memset(o64m, 1.0 / DK)
            eps_t = P.tile([1, 1], F32, tag="c_e", bufs=1)
            nc.vector.memset(eps_t, EPS)

            gate("const")
            # ---------------- relative position bias ----------------
            # rbar[p] = mean_d rel_emb[p, d]
            # pos_T[jt][j, i] = rbar[511 - (128*jt + j) + i]
            rel_sb = P.tile([DK, 2 * MAXPOS - 1], F32, tag="pre", bufs=3)
            nc.sync.dma_start(out=rel_sb, in_=rel_embT[:, :])
            rbar_sb = P.tile([1, 2 * MAXPOS - 1], F32, tag="pre", bufs=3)
            for c0, w in ((0, 512), (512, 511)):
                pr = Q.tile([1, 512], F32, tag="s", bufs=2)
                MM(pr[:, :w], o64m, rel_sb[:, c0:c0 + w], start=True, stop=True)
                nc.scalar.activation(out=rbar_sb[:, c0:c0 + w], in_=pr[:, :w],
                                     func=AF.Copy)
            nc.sync.dma_start(out=rbar_dram.ap().unsqueeze(0), in_=rbar_sb[0:1, :])

            J_sb = P.tile([128, 128], F32, tag="jrev", bufs=1)
            nc.sync.dma_start(out=J_sb, in_=Jx[:, :])
            pos_T = []
            for jt in range(4):
                A_t = P.tile([128, 512], F32, tag="pre", bufs=3)
                src = bass.AP(tensor=rbar_dram.ap().tensor,
                              offset=384 - 128 * jt, ap=[[1, 128], [1, 512]])
                nc.sync.dma_start(out=A_t, in_=src)
                pp = Q.tile([128, 512], F32, tag="fc", bufs=2)
                MM(pp, J_sb, A_t, start=True, stop=True)
                pt = P.tile([128, 512], F32, tag="posT", bufs=4)
                nc.vector.tensor_copy(pt, pp)
                pos_T.append(pt)

            gate("pos")
            # -------- persistent v (token-major, ones-augmented) --------
            vv = []
            for rt in range(8):
                t = P.tile([128, H * (DK + 1)], F32R, tag="vv", bufs=8)
                v3 = t.rearrange("p (h e) -> p h e", e=DK + 1)
                nc.vector.tensor_copy(v3[:, :, DK:DK + 1], ones8.unsqueeze(2))
                vv.append(t)

            # -------- big activation buffer allocator (4 rotating tags) ----
            free_tags = ["bA", "bB", "bC", "bD"]

            def alloc_act():
                tag = free_tags.pop(0)
                tiles = [P.tile([128, R], F32R, tag=tag, bufs=4,
                                name=f"{tag}_{nc.next_id()}")
                         for _ in range(DT)]
                return tiles, tag

            def free_act(tag):
                free_tags.append(tag)

            # ---------------- input projection -> xT ----------------
            xin_sb = P.tile([FC, R], F32R, tag="pre", bufs=3)
            nc.sync.dma_start(out=xin_sb, in_=x_cgmT[:, :])
            cgmW_sb = P.tile([FC, D], F32R, tag="pre", bufs=3)
            nc.sync.dma_start(out=cgmW_sb, in_=cgm_W[:, :])
            cgmb_sb = P.tile([128, DT], F32, tag="b_cgm", bufs=2)
            for t_ in range(DT):
                nc.sync.dma_start(out=cgmb_sb[:, t_:t_ + 1],
                                  in_=cgm_b[t_ * 128:(t_ + 1) * 128].unsqueeze(1))

            xT, xT_tag = alloc_act()
            for t_ in range(DT):
                for rc in range(RC):
                    ps = Q.tile([128, 512], F32, tag="acc", bufs=4)
                    MM(ps, cgmW_sb[:, t_ * 128:(t_ + 1) * 128],
                       xin_sb[:, rc * 512:(rc + 1) * 512], start=True, stop=True)
                    nc.scalar.activation(
                        out=xT[t_][:, rc * 512:(rc + 1) * 512], in_=ps,
                        func=AF.Identity, bias=cgmb_sb[:, t_:t_ + 1])

            gate("xT")
            # ---------------- helpers ----------------
            def load_bias(dram, ncols, tag):
                bt = P.tile([128, ncols], F32, tag=tag, bufs=2)
                for t_ in range(ncols):
                    nc.sync.dma_start(out=bt[:, t_:t_ + 1],
                                      in_=dram[t_ * 128:(t_ + 1) * 128].unsqueeze(1))
                return bt

            def proj(dst, w_dram, b_sb, src, act=AF.Identity):
                """dst[nt] (feature-major) = act(src @ W + b); W [512, 512]."""
                wts = []
                for kt in range(DT):
                    wt = P.tile([128, 512], F32R, tag="w512", bufs=8)
                    nc.sync.dma_start(out=wt, in_=w_dram[kt * 128:(kt + 1) * 128, :])
                    wts.append(wt)
                for nt in range(DT):
                    for rc in range(RC):
                        ps = Q.tile([128, 512], F32, tag="acc", bufs=4)
                        for kt in range(DT):
                            MM(ps, wts[kt][:, nt * 128:(nt + 1) * 128],
                               src[kt][:, rc * 512:(rc + 1) * 512],
                               start=(kt == 0), stop=(kt == DT - 1))
                        nc.scalar.activation(
                            out=dst[nt][:, rc * 512:(rc + 1) * 512], in_=ps,
                            func=act, bias=b_sb[:, nt:nt + 1])

            def layernorm(res, s_sb, b_sb, dst):
                """dst = LN(res) over the partition(feature) axis."""
                for rc in range(RC):
                    sl = slice(rc * 512, (rc + 1) * 512)
                    s1p = Q.tile([1, 512], F32, tag="s", bufs=2)
                    for kt in range(DT):
                        MM(s1p, ones_col, res[kt][:, sl],
                           start=(kt == 0), stop=(kt == DT - 1))
                    s2p = Q.tile([1, 512], F32, tag="s", bufs=2)
                    for kt in range(DT):
                        sq_t = P.tile([128, 512], F32R, tag="sqc", bufs=5)
                        nc.vector.tensor_mul(sq_t, res[kt][:, sl],
                                             res[kt][:, sl])
                        MM(s2p, ones_col, sq_t,
                           start=(kt == 0), stop=(kt == DT - 1))
                    mu = P.tile([1, 512], F32R, tag="ln_mu", bufs=4)
                    nc.scalar.activation(out=mu, in_=s1p, func=AF.Copy,
                                         scale=1.0 / D)
                    s2m = P.tile([1, 512], F32, tag="ln_t", bufs=4)
                    nc.scalar.activation(out=s2m, in_=s2p, func=AF.Copy,
                                         scale=1.0 / D)
                    mu2 = P.tile([1, 512], F32, tag="ln_t", bufs=4)
                    nc.vector.tensor_mul(mu2, mu, mu)
                    var = P.tile([1, 512], F32, tag="ln_t", bufs=4)
                    nc.vector.tensor_tensor(var, s2m, mu2, OP.subtract)
                    sq = P.tile([1, 512], F32, tag="ln_t", bufs=4)
                    nc.scalar.activation(out=sq, in_=var, func=AF.Sqrt,
                                         bias=eps_t)
                    rs = P.tile([1, 512], F32R, tag="ln_mu", bufs=4)
                    nc.vector.reciprocal(rs, sq)
                    mub = Q.tile([128, 512], F32, tag="fc", bufs=2)
                    MM(mub, ones_row, mu, start=True, stop=True)
                    rsb = Q.tile([128, 512], F32, tag="fc", bufs=2)
                    MM(rsb, ones_row, rs, start=True, stop=True)
                    for kt in range(DT):
                        t1 = P.tile([128, 512], F32, tag="tmp", bufs=4)
                        nc.vector.tensor_tensor(t1, res[kt][:, sl], mub,
                                                OP.subtract)
                        t2 = P.tile([128, 512], F32, tag="tmp", bufs=4)
                        nc.vector.scalar_tensor_tensor(
                            t2, t1, s_sb[:, kt:kt + 1], rsb,
                            op0=OP.mult, op1=OP.mult)
                        nc.scalar.activation(out=dst[kt][:, sl], in_=t2,
                                             func=AF.Identity,
                                             bias=b_sb[:, kt:kt + 1])

            # ---------------- transformer layers ----------------
            for l in range(int(os.environ.get('KLAYERS', L))):
                bq_sb = load_bias(bq[l], DT, "b_q")
                bk_sb = load_bias(bk[l], DT, "b_k")
                bo_sb = load_bias(bo[l], DT, "b_o")
                bg_sb = load_bias(bg[l], DT, "b_g")
                bf1_sb = load_bias(bf1[l], FT, "b_f1")
                bfg_sb = load_bias(bfg[l], FT, "b_fg")
                bf2_sb = load_bias(bf2[l], DT, "b_f2")
                l1s_sb = load_bias(ln1_s[l], DT, "b_l1s")
                l1b_sb = load_bias(ln1_b[l], DT, "b_l1b")
                l2s_sb = load_bias(ln2_s[l], DT, "b_l2s")
                l2b_sb = load_bias(ln2_b[l], DT, "b_l2b")
                bvf = P.tile([1, HDK], F32, tag="b_vf", bufs=2)
                nc.sync.dma_start(out=bvf, in_=bv[l].unsqueeze(0))
                bv_row = P.tile([1, HDK], F32R, tag="b_vr", bufs=2)
                nc.vector.tensor_copy(bv_row, bvf)

                qT, qT_tag = alloc_act()
                proj(qT, Wq[l], bq_sb, xT)
                kTt, kT_tag = alloc_act()
                proj(kTt, Wk[l], bk_sb, xT)

                # v token-major with bias via ones-row matmul
                wv_sb = []
                for kt in range(DT):
                    wt = P.tile([128, HDK], F32R, tag="w512", bufs=8)
                    nc.sync.dma_start(out=wt,
                                      in_=Wv[l][kt * 128:(kt + 1) * 128, :])
                    wv_sb.append(wt)
                for rt in range(8):
                    ps = Q.tile([128, 512], F32, tag="acc", bufs=4)
                    for kt in range(DT):
                        MM(ps, xT[kt][:, rt * 128:(rt + 1) * 128], wv_sb[kt],
                           start=(kt == 0), stop=False)
                    MM(ps, ones_row[:, :128], bv_row, start=False, stop=True)
                    v3 = vv[rt].rearrange("p (h e) -> p h e", e=DK + 1)
                    nc.scalar.activation(
                        out=v3[:, :, 0:DK],
                        in_=ps.rearrange("p (h d) -> p h d", d=DK),
                        func=AF.Copy)

                # attention (scores-transposed softmax)
                ctxT, ctx_tag = alloc_act()
                for b in range(BL):
                    for hp in range(4):
                        pb = [[None] * 4 for _ in range(2)]  # noqa
                        for jt in range(4):
                            for h01 in range(2):
                                hs = slice(h01 * 64, h01 * 64 + 64)
                                ps = Q.tile([128, 512], F32, tag="acc", bufs=4)
                                MM(ps,
                                   kTt[hp][hs, b * 512 + jt * 128:
                                           b * 512 + jt * 128 + 128],
                                   qT[hp][hs, b * 512:(b + 1) * 512],
                                   start=True, stop=True)
                                tm = P.tile([128, 512], F32, tag="tmp", bufs=4)
                                nc.vector.scalar_tensor_tensor(
                                    tm, ps, 0.125, pos_T[jt],
                                    op0=OP.mult, op1=OP.add)
                                pr = P.tile([128, 512], F32R, tag="probs",
                                            bufs=8)
                                nc.scalar.activation(out=pr, in_=tm,
                                                     func=AF.Exp)
                                pb[h01][jt] = pr
                        for h01 in range(2):
                            h = hp * 2 + h01
                            pc = Q.tile([DK + 1, 512], F32, tag="fc", bufs=2)
                            for jt in range(4):
                                MM(pc,
                                   vv[b * 4 + jt][:, h * (DK + 1):
                                                  (h + 1) * (DK + 1)],
                                   pb[h01][jt],
                                   start=(jt == 0), stop=(jt == 3))
                            rden = P.tile([1, 512], F32R, tag="rden", bufs=2)
                            nc.vector.reciprocal(rden, pc[DK:DK + 1, :])
                            pbc = Q.tile([64, 512], F32, tag="s", bufs=2)
                            MM(pbc, ones_row[:, :64], rden,
                               start=True, stop=True)
                            ctmp = P.tile([64, 512], F32, tag="ctmp", bufs=2)
                            nc.scalar.activation(out=ctmp, in_=pc[0:DK, :],
                                                 func=AF.Copy)
                            nc.vector.tensor_mul(
                                ctxT[hp][h01 * 64:h01 * 64 + 64,
                                         b * 512:(b + 1) * 512],
                                ctmp, pbc)
                free_act(qT_tag)   # qT dead after scores
                gT, gT_tag = alloc_act()
                proj(gT, Wg[l], bg_sb, xT, act=AF.Sigmoid)
                free_act(kT_tag)   # kT dead after scores
                attT, attT_tag = alloc_act()
                proj(attT, Wo[l], bo_sb, ctxT)
                free_act(ctx_tag)

                # res = x + gate * att ; x1 = LN1(res)
                res, res_tag = alloc_act()
                for kt in range(DT):
                    for rc in range(RC):
                        sl = slice(rc * 512, (rc + 1) * 512)
                        tm = P.tile([128, 512], F32, tag="tmp", bufs=4)
                        nc.vector.tensor_mul(tm, gT[kt][:, sl],
                                             attT[kt][:, sl])
                        nc.vector.tensor_add(res[kt][:, sl], tm,
                                             xT[kt][:, sl])
                free_act(xT_tag)
                free_act(gT_tag)
                free_act(attT_tag)
                x1, x1_tag = alloc_act()
                layernorm(res, l1s_sb, l1b_sb, x1)
                free_act(res_tag)

                # FF: f = (x1@Wf1 + bf1) * sigmoid(x1@Wfg + bfg); ff = f@Wf2
                res2, res2_tag = alloc_act()
                for rc in range(RC):
                    sl = slice(rc * 512, (rc + 1) * 512)
                    accs = [Q.tile([128, 512], F32, tag="acc", bufs=4,
                                   name=f"acc_{nc.next_id()}")
                            for _ in range(DT)]
                    for ntg in range(4):
                        wf1g, wfgg = [], []
                        for kt in range(DT):
                            w1 = P.tile([128, 512], F32R, tag="w512", bufs=8)
                            nc.sync.dma_start(
                                out=w1, in_=Wf1[l][kt * 128:(kt + 1) * 128,
                                                   ntg * 512:(ntg + 1) * 512])
                            wf1g.append(w1)
                            wg_ = P.tile([128, 512], F32R, tag="w512", bufs=8)
                            nc.sync.dma_start(
                                out=wg_, in_=Wfg[l][kt * 128:(kt + 1) * 128,
                                                    ntg * 512:(ntg + 1) * 512])
                            wfgg.append(wg_)
                        for ntl in range(4):
                            nt = ntg * 4 + ntl
                            nsl = slice(ntl * 128, (ntl + 1) * 128)
                            p1 = Q.tile([128, 512], F32, tag="fc", bufs=2)
                            for kt in range(DT):
                                MM(p1, wf1g[kt][:, nsl], x1[kt][:, sl],
                                   start=(kt == 0), stop=(kt == DT - 1))
                            pg = Q.tile([128, 512], F32, tag="fc", bufs=2)
                            for kt in range(DT):
                                MM(pg, wfgg[kt][:, nsl], x1[kt][:, sl],
                                   start=(kt == 0), stop=(kt == DT - 1))
                            sg = P.tile([128, 512], F32, tag="sg", bufs=2)
                            nc.scalar.activation(out=sg, in_=pg,
                                                 func=AF.Sigmoid,
                                                 bias=bfg_sb[:, nt:nt + 1])
                            ft = P.tile([128, 512], F32R, tag="ft", bufs=3)
                            nc.vector.scalar_tensor_tensor(
                                ft, p1, bf1_sb[:, nt:nt + 1], sg,
                                op0=OP.add, op1=OP.mult)
                            wf2t = P.tile([128, 512], F32R, tag="wf2", bufs=4)
                            nc.sync.dma_start(
                                out=wf2t,
                                in_=Wf2[l][nt * 128:(nt + 1) * 128, :])
                            for dt_ in range(DT):
                                MM(accs[dt_],
                                   wf2t[:, dt_ * 128:(dt_ + 1) * 128],
                                   ft, start=(nt == 0), stop=(nt == FT - 1))
                    for dt_ in range(DT):
                        nc.vector.scalar_tensor_tensor(
                            res2[dt_][:, sl], accs[dt_],
                            bf2_sb[:, dt_:dt_ + 1],
                            x1[dt_][:, sl], op0=OP.add, op1=OP.add)
                free_act(x1_tag)
                xT, xT_tag = alloc_act()
                layernorm(res2, l2s_sb, l2b_sb, xT)
                free_act(res2_tag)

            gate("layers")
            # ---------------- head ----------------
            hT = []
            for kt in range(DT):
                xr = P.tile([128, BL], F32, tag="hd", bufs=8)
                nc.vector.tensor_reduce(
                    xr, xT[kt].rearrange("p (b s) -> p b s", b=BL),
                    axis=mybir.AxisListType.X, op=OP.add)
                ht = P.tile([128, BL], F32R, tag="hT", bufs=8)
                nc.scalar.activation(out=ht, in_=xr, func=AF.Copy,
                                     scale=1.0 / S)
                hT.append(ht)
            ow_sb = P.tile([FO, D], F32R, tag="ow", bufs=1)
            nc.sync.dma_start(out=ow_sb, in_=other_W[:, :])
            ob_sb = load_bias(other_b, DT, "b_ob")
            xo_sb = P.tile([FO, BL], F32R, tag="xo", bufs=1)
            nc.sync.dma_start(out=xo_sb, in_=x_otherT[:, :])
            for nt in range(DT):
                ps = Q.tile([128, BL], F32, tag="acc", bufs=4)
                MM(ps, ow_sb[:, nt * 128:(nt + 1) * 128], xo_sb,
                   start=True, stop=True)
                ht = P.tile([128, BL], F32R, tag="hT", bufs=8)
                nc.scalar.activation(out=ht, in_=ps, func=AF.Identity,
                                     bias=ob_sb[:, nt:nt + 1])
                hT.append(ht)

            def head_ln_relu(zt, n_tiles, nfeat, s_sb, b_sb, outtag):
                s1p = Q.tile([1, BL], F32, tag="s", bufs=2)
                for kt in range(n_tiles):
                    MM(s1p, ones_col, zt[kt], start=(kt == 0),
                       stop=(kt == n_tiles - 1))
                s2p = Q.tile([1, BL], F32, tag="s", bufs=2)
                for kt in range(n_tiles):
                    z2 = P.tile([128, BL], F32R, tag="hd2", bufs=4)
                    nc.vector.tensor_mul(z2, zt[kt], zt[kt])
                    MM(s2p, ones_col, z2, start=(kt == 0),
                       stop=(kt == n_tiles - 1))
                mu = P.tile([1, BL], F32R, tag="hmu", bufs=4)
                nc.scalar.activation(out=mu, in_=s1p, func=AF.Copy,
                                     scale=1.0 / nfeat)
                s2m = P.tile([1, BL], F32, tag="hln", bufs=8)
                nc.scalar.activation(out=s2m, in_=s2p, func=AF.Copy,
                                     scale=1.0 / nfeat)
                mu2 = P.tile([1, BL], F32, tag="hln", bufs=8)
                nc.vector.tensor_mul(mu2, mu, mu)
                var = P.tile([1, BL], F32, tag="hln", bufs=8)
                nc.vector.tensor_tensor(var, s2m, mu2, OP.subtract)
                sq = P.tile([1, BL], F32, tag="hln", bufs=8)
                nc.scalar.activation(out=sq, in_=var, func=AF.Sqrt, bias=eps_t)
                rs = P.tile([1, BL], F32R, tag="hmu", bufs=4)
                nc.vector.reciprocal(rs, sq)
                mub = Q.tile([128, BL], F32, tag="fc", bufs=2)
                MM(mub, ones_row, mu, start=True, stop=True)
                rsb = Q.tile([128, BL], F32, tag="fc", bufs=2)
                MM(rsb, ones_row, rs, start=True, stop=True)
                outs = []
                for kt in range(n_tiles):
                    t1 = P.tile([128, BL], F32, tag="hd", bufs=8)
                    nc.vector.tensor_tensor(t1, zt[kt], mub, OP.subtract)
                    t2 = P.tile([128, BL], F32, tag="hd", bufs=8)
                    nc.vector.scalar_tensor_tensor(
                        t2, t1, s_sb[:, kt:kt + 1], rsb,
                        op0=OP.mult, op1=OP.mult)
                    o = P.tile([128, BL], F32R, tag=outtag, bufs=4)
                    nc.scalar.activation(out=o, in_=t2, func=AF.Relu,
                                         bias=b_sb[:, kt:kt + 1])
                    outs.append(o)
                return outs

            # fc1 [1024 -> 256]
            fw1_sb = []
            for kt in range(8):
                wt = P.tile([128, 256], F32R, tag="w512", bufs=8)
                nc.sync.dma_start(out=wt, in_=fW1[kt * 128:(kt + 1) * 128, :])
                fw1_sb.append(wt)
            fb1_sb = load_bias(fb1, 2, "b_fb1")
            f1s_sb = load_bias(fln1_s, 2, "b_fl1s")
            f1b_sb = load_bias(fln1_b, 2, "b_fl1b")
            z1 = []
            for nt in range(2):
                ps = Q.tile([128, BL], F32, tag="acc", bufs=4)
                for kt in range(8):
                    MM(ps, fw1_sb[kt][:, nt * 128:(nt + 1) * 128], hT[kt],
                       start=(kt == 0), stop=(kt == 7))
                z = P.tile([128, BL], F32R, tag="z1", bufs=2)
                nc.scalar.activation(out=z, in_=ps, func=AF.Identity,
                                     bias=fb1_sb[:, nt:nt + 1])
                z1.append(z)
            h1 = head_ln_relu(z1, 2, 256, f1s_sb, f1b_sb, "h1")

            # fc2 [256 -> 128]
            fw2_sb = []
            for kt in range(2):
                wt = P.tile([128, 128], F32R, tag="w512", bufs=8)
                nc.sync.dma_start(out=wt, in_=fW2[kt * 128:(kt + 1) * 128, :])
                fw2_sb.append(wt)
            fb2_sb = load_bias(fb2, 1, "b_fb2")
            f2s_sb = load_bias(fln2_s, 1, "b_fl2s")
            f2b_sb = load_bias(fln2_b, 1, "b_fl2b")
            ps = Q.tile([128, BL], F32, tag="acc", bufs=4)
            for kt in range(2):
                MM(ps, fw2_sb[kt], h1[kt], start=(kt == 0), stop=(kt == 1))
            z2_ = P.tile([128, BL], F32R, tag="z2", bufs=2)
            nc.scalar.activation(out=z2_, in_=ps, func=AF.Identity,
                                 bias=fb2_sb[:, 0:1])
            h2 = head_ln_relu([z2_], 1, 128, f2s_sb, f2b_sb, "h2")

            # fc3 [128 -> 1]
            fw3_sb = P.tile([128, 1], F32R, tag="fw3", bufs=1)
            nc.sync.dma_start(out=fw3_sb, in_=fW3[:, :])
            fb3_sb = P.tile([1, 1], F32, tag="fb3", bufs=1)
            nc.sync.dma_start(out=fb3_sb, in_=fb3.ap().unsqueeze(0))
            ps = Q.tile([1, BL], F32, tag="s", bufs=2)
            MM(ps, fw3_sb, h2[0], start=True, stop=True)
            out_sb = P.tile([1, BL], F32, tag="outsb", bufs=1)
            nc.scalar.activation(out=out_sb, in_=ps, func=AF.Identity,
                                 bias=fb3_sb)
            nc.sync.dma_start(out=out_ext[:, :], in_=out_sb)
          except _Stop:
            pass

    nc.compile()
    return nc


def _get_nc():
    if "nc" not in _CACHE:
        _CACHE["nc"] = _build()
    return _CACHE["nc"]


def kernel(**inputs):
    np32 = lambda a: np.ascontiguousarray(np.asarray(a, dtype=np.float32))
    shared = {
        "cgm_W": np32(inputs["cgm_W"]),
        "cgm_b": np32(inputs["cgm_b"]),
        "rel_embT": np32(np.asarray(inputs["rel_emb"], np.float32).T),
        "J": np.eye(128, dtype=np.float32)[::-1].copy(),
        "other_W": np32(inputs["other_W"]),
        "other_b": np32(inputs["other_b"]),
        "fW1": np32(inputs["fW1"]), "fb1": np32(inputs["fb1"]),
        "fln1_s": np32(inputs["fln1_s"]), "fln1_b": np32(inputs["fln1_b"]),
        "fW2": np32(inputs["fW2"]), "fb2": np32(inputs["fb2"]),
        "fln2_s": np32(inputs["fln2_s"]), "fln2_b": np32(inputs["fln2_b"]),
        "fW3": np32(inputs["fW3"]), "fb3": np32(inputs["fb3"]),
    }
    for nm in ("Wq", "bq", "Wk", "bk", "Wv", "bv", "Wo", "bo", "Wg", "bg",
               "Wf1", "bf1", "Wfg", "bfg", "Wf2", "bf2",
               "ln1_s", "ln1_b", "ln2_s", "ln2_b"):
        shared[nm] = np32(inputs[nm])

    x_cgm = np.asarray(inputs["x_cgm"], np.float32)
    x_other = np.asarray(inputs["x_other"], np.float32)
    in_maps = []
    for c in range(NCORES):
        m = dict(shared)
        xs = x_cgm[c * BL:(c + 1) * BL].reshape(R, FC).T
        m["x_cgmT"] = np.ascontiguousarray(xs)
        m["x_otherT"] = np.ascontiguousarray(x_other[c * BL:(c + 1) * BL].T)
        in_maps.append(m)

    nc = _get_nc()
    trace = bool(int(os.environ.get("KTRACE", "0")))
    res = run_bass_kernel_spmd(nc, in_maps, core_ids=list(range(NCORES)),
                               trace=trace)
    _CACHE["last_res"] = res
    out = np.concatenate(
        [res.results[c]["out"].reshape(BL, 1) for c in range(NCORES)], axis=0)
    return out.astype(np.float32)



# revision 14
# speedup vs baseline: 1.6462x; 1.6462x over previous
"""Trainium2 Bass kernel for nn_AttentionModel (4-layer gated transformer).

Sharding: pure data-parallel over batch (B=16 -> 2 per core, 8 cores, no
collectives). Feature-major activations in bf16 (fp32 PSUM accumulate).

Perf structure:
- Weights host-packed to bf16 blobs; one DMA per layer stage (attn 2.6MB,
  ff 6.3MB), prefetched a stage ahead. All biases/LN params in one fp32
  [128, NP] blob -> single DMA.
- All activation functions from the exp_and_others table set (exp/tanh/
  identity/copy/square) except LN Sqrt: sigmoid(z) folded to
  (tanh(z/2)+1) * 0.5 with the 0.5s folded into weights host-side.
- Softmax: probs = exp(s)*exp(p); exp(pos_bias) precomputed on host (bf16
  multiplicative factor) so scores go PSUM -ACT-> exp -TT-> probs (bf16).
  Score scale 1/8 folded into Wq. Denominators via ones-augmented V column;
  reciprocal_approx_fast (~5x faster than vector.reciprocal).
- LayerNorm: partition sums via PE ones-matmuls, batched stats for both
  512-token chunks in one [2,512] tile, recip_approx for 1/sigma.
"""

import os
import sys

for _p in ("/opt/trn_rl_repo",):
    if os.path.isdir(_p) and _p not in sys.path:
        sys.path.insert(0, _p)

import numpy as np
import ml_dtypes

import concourse.bass as bass
import concourse.mybir as mybir
import concourse.tile as tile
from concourse import bacc
from concourse.bass_utils import run_bass_kernel_spmd

F32 = mybir.dt.float32
F32R = mybir.dt.float32r
BF = mybir.dt.bfloat16
NPBF = ml_dtypes.bfloat16
AF = mybir.ActivationFunctionType
OP = mybir.AluOpType

B, S, FC, FO = 16, 512, 24, 16
D, H, DK, FFD, L = 512, 8, 64, 2048, 4
MAXPOS = 512
EPS = 1e-6

NCORES = 8
BL = B // NCORES          # local batch = 2
R = BL * S                # local tokens = 1024
DT = D // 128             # feature tiles = 4
FT = FFD // 128           # ff tiles = 16
HDK = H * DK

# aw blob column bases (per layer, [128, 10240] bf16)
AW_Q, AW_K, AW_V, AW_O, AW_G = 0, 2048, 4096, 6144, 8192
AW_COLS = 10240
# fw blob column bases ([128, 24576] bf16)
FW_1, FW_G, FW_2 = 0, 8192, 16384
FW_COLS = 24576
# param blob columns (fp32 [128, NP])
PL = 68                   # per-layer stride
# per-layer: bq 0, bk 4, bo 8, bg 12, l1s 16, l1b 20, l2s 24, l2b 28,
#            bf1 32, bfg 48, bf2 64
HB = L * PL               # head base = 272
# head: cgm_b +0, other_b +4, fb1 +8, fl1s +10, fl1b +12, fb2 +14,
#       fl2s +15, fl2b +16, fw3 +17, fb3 +18 (row 0)
NP = HB + 19

_CACHE = {}


def _build():
    nc = bacc.Bacc("TRN2", target_bir_lowering=False, debug=False,
                   num_devices=NCORES)

    def par(name, shape, dt):
        return nc.declare_dram_parameter(name, list(shape), dt, isOutput=False)

    xin_d = par("xin", [FC, R], BF)
    xo_d = par("xo", [FO, BL], BF)
    cgmW_d = par("cgmW", [FC, D], BF)
    posE_d = par("posE", [128, 4 * 512], BF)
    aw_d = par("aw", [L, 128, AW_COLS], BF)
    fw_d = par("fw", [L, 128, FW_COLS], BF)
    pb_d = par("pb", [128, NP], F32)
    bvr_d = par("bvr", [L, HDK], F32R)
    hw1_d = par("hw1", [128, 8 * 256], BF)
    hw2_d = par("hw2", [128, 2 * 128 + 1], BF)
    ow_d = par("ow", [FO, D], BF)
    out_ext = nc.declare_dram_parameter("out", [1, BL], F32, isOutput=True)

    with tile.TileContext(nc) as tc:
        with (
            nc.allow_low_precision(reason="bf16 matmul/activation pipeline"),
            tc.tile_pool(name="P", bufs=1) as P,
            tc.tile_pool(name="Q", bufs=1, space="PSUM") as Q,
        ):
            MM = nc.tensor.matmul
            NLAYERS = int(os.environ.get("KLAYERS", L))
            KPROBE = os.environ.get("KPROBE", "")
            if KPROBE:
                dbg_ext = nc.declare_dram_parameter(
                    "dbg", [128, 1024], F32, isOutput=True)
                dbg_done = [False]

                def probe(name, ap):
                    if name != KPROBE or dbg_done[0]:
                        return
                    dbg_done[0] = True
                    pdim = ap.shape[0]
                    fdim = ap.free_size()
                    dt_ = P.tile([128, 1024], F32, tag="dbgt", bufs=1)
                    nc.vector.memset(dt_, 0.0)
                    nc.vector.tensor_copy(
                        dt_[0:pdim, 0:fdim], ap)
                    nc.sync.dma_start(out=dbg_ext[:, :], in_=dt_)
            else:
                def probe(name, ap):
                    pass

            # ---------------- constants ----------------
            ones_col = P.tile([128, 1], BF, tag="c_oc", bufs=1)
            nc.vector.memset(ones_col, 1.0)
            ones_row_f = P.tile([1, 128], F32, tag="c_orf", bufs=1)
            nc.vector.memset(ones_row_f, 1.0)
            ones_row_r = P.tile([1, 128], F32R, tag="c_orr", bufs=1)
            nc.vector.tensor_copy(ones_row_r, ones_row_f)
            eps2 = P.tile([2, 1], F32, tag="c_e", bufs=1)
            nc.vector.memset(eps2, EPS)

            # ---------------- persistent loads ----------------
            pb_sb = P.tile([128, NP], F32, tag="pb", bufs=1)
            nc.sync.dma_start(out=pb_sb, in_=pb_d[:, :])
            posE_sb = P.tile([128, 2048], BF, tag="posE", bufs=1)
            nc.sync.dma_start(out=posE_sb, in_=posE_d[:, :])
            xin_sb = P.tile([FC, R], BF, tag="xin", bufs=1)
            nc.sync.dma_start(out=xin_sb, in_=xin_d[:, :])
            cgmW_sb = P.tile([FC, D], BF, tag="cgmW", bufs=1)
            nc.sync.dma_start(out=cgmW_sb, in_=cgmW_d[:, :])
            xo_sb = P.tile([FO, BL], BF, tag="xo", bufs=1)
            nc.sync.dma_start(out=xo_sb, in_=xo_d[:, :])
            ow_sb = P.tile([FO, D], BF, tag="ow", bufs=1)
            nc.sync.dma_start(out=ow_sb, in_=ow_d[:, :])
            hw1_sb = P.tile([128, 2048], BF, tag="hw1", bufs=1)
            nc.sync.dma_start(out=hw1_sb, in_=hw1_d[:, :])
            hw2_sb = P.tile([128, 257], BF, tag="hw2", bufs=1)
            nc.sync.dma_start(out=hw2_sb, in_=hw2_d[:, :])
            bvr_sb = []
            for l in range(L):
                t = P.tile([1, HDK], F32R, tag="bvr", bufs=L,
                           name=f"bvr{l}")
                nc.sync.dma_start(out=t, in_=bvr_d[l].unsqueeze(0))
                bvr_sb.append(t)

            def col(c, n=1):
                return pb_sb[:, c:c + n]

            # layer weight pools
            def load_aw(l):
                t = P.tile([128, AW_COLS], BF, tag="aw",
                           bufs=(1 if KPROBE else 2),
                           name=f"aw{l}")
                nc.sync.dma_start(out=t, in_=aw_d[l])
                return t

            def load_fw(l):
                t = P.tile([128, FW_COLS], BF, tag="fw", bufs=1,
                           name=f"fw{l}")
                nc.sync.dma_start(out=t, in_=fw_d[l])
                return t

            aw_sb = load_aw(0)

            # ------------- activation tile allocator -------------
            free_tags = ["bA", "bB", "bC", "bD", "bE", "bF"]

            def alloc_act():
                tag = free_tags.pop(0)
                tiles = [P.tile([128, R], BF, tag=tag, bufs=4,
                                name=f"{tag}_{nc.next_id()}")
                         for _ in range(DT)]
                return tiles, tag

            def free_act(tag):
                free_tags.append(tag)

            # persistent token-major V (ones-augmented)
            vv = []
            for rt in range(8):
                t = P.tile([128, H * (DK + 1)], BF, tag="vv", bufs=8,
                           name=f"vv{rt}")
                v3 = t.rearrange("p (h e) -> p h e", e=DK + 1)
                nc.vector.memset(v3[:, :, DK:DK + 1], 1.0)
                vv.append(t)

            # ---------------- input projection ----------------
            xT, xT_tag = alloc_act()
            for nt in range(DT):
                for rc in range(2):
                    ps = Q.tile([128, 512], F32, tag="B", bufs=2)
                    MM(ps, cgmW_sb[:, nt * 128:(nt + 1) * 128],
                       xin_sb[:, rc * 512:(rc + 1) * 512],
                       start=True, stop=True)
                    nc.scalar.activation(
                        out=xT[nt][:, rc * 512:(rc + 1) * 512], in_=ps,
                        func=AF.Identity, bias=col(HB + nt))

            # ---------------- helpers ----------------
            def proj_v(dst, wbase, bcols, src, act=None):
                """dst[nt] = act(src @ W + b), feature-major.

                act=None: vector tensor_scalar add-bias (PSUM->bf16)
                act=AF.*: scalar activation with bias
                """
                for nt in range(DT):
                    for rc in range(2):
                        ps = Q.tile([128, 512], F32, tag="B", bufs=2,
                                    name=f"pj_{nc.next_id()}")
                        for kt in range(DT):
                            MM(ps,
                               aw_sb[:, wbase + kt * 512 + nt * 128:
                                     wbase + kt * 512 + nt * 128 + 128],
                               src[kt][:, rc * 512:(rc + 1) * 512],
                               start=(kt == 0), stop=(kt == DT - 1))
                        o = dst[nt][:, rc * 512:(rc + 1) * 512]
                        if act is None:
                            nc.vector.tensor_scalar(
                                out=o, in0=ps, scalar1=col(bcols + nt),
                                scalar2=None, op0=OP.add)
                        else:
                            nc.scalar.activation(out=o, in_=ps, func=act,
                                                 bias=col(bcols + nt))

            def layernorm(res, cs, cb, dst):
                """dst = LN(res) over features (partitions)."""
                for rc in range(2):
                    sl = slice(rc * 512, (rc + 1) * 512)
                    s1p = Q.tile([1, 512], F32, tag="B", bufs=2,
                                 name=f"s1_{nc.next_id()}")
                    s2p = Q.tile([1, 512], F32, tag="C", bufs=2,
                                 name=f"s2_{nc.next_id()}")
                    for kt in range(DT):
                        MM(s1p, ones_col, res[kt][:, sl],
                           start=(kt == 0), stop=(kt == DT - 1))
                    for kt in range(DT):
                        sq = P.tile([128, 512], BF, tag="scr", bufs=5,
                                    name=f"sq_{nc.next_id()}")
                        nc.vector.tensor_mul(sq, res[kt][:, sl],
                                             res[kt][:, sl])
                        MM(s2p, ones_col, sq,
                           start=(kt == 0), stop=(kt == DT - 1))
                    mu = P.tile([1, 512], F32R, tag="ln_mu", bufs=3,
                                name=f"mu_{nc.next_id()}")
                    nc.vector.tensor_scalar(out=mu, in0=s1p,
                                            scalar1=1.0 / D,
                                            scalar2=None, op0=OP.mult)
                    m2 = P.tile([1, 512], F32, tag="ln_t", bufs=3,
                                name=f"m2_{nc.next_id()}")
                    nc.vector.tensor_scalar(out=m2, in0=s2p,
                                            scalar1=1.0 / D,
                                            scalar2=None, op0=OP.mult)
                    var = P.tile([1, 512], F32, tag="ln_t", bufs=3,
                                 name=f"va_{nc.next_id()}")
                    nc.vector.scalar_tensor_tensor(
                        var, mu, -1.0, mu, op0=OP.mult, op1=OP.mult)
                    nc.vector.tensor_add(var, var, m2)
                    sg = P.tile([1, 512], F32, tag="ln_t", bufs=3,
                                name=f"sg_{nc.next_id()}")
                    nc.scalar.activation(out=sg, in_=var, func=AF.Sqrt,
                                         bias=eps2[0:1, :])
                    rs = P.tile([1, 512], F32, tag="ln_mu", bufs=3,
                                name=f"rs_{nc.next_id()}")
                    nc.vector.reciprocal_approx_fast(out=rs, in_=sg)
                    rsr = P.tile([1, 512], F32R, tag="ln_mu", bufs=3,
                                 name=f"rsr_{nc.next_id()}")
                    nc.vector.tensor_copy(rsr, rs)
                    mub = Q.tile([128, 512], F32, tag="C", bufs=2,
                                 name=f"mb_{nc.next_id()}")
                    MM(mub, ones_row_r, mu, start=True, stop=True)
                    rsb = Q.tile([128, 512], F32, tag="B", bufs=2,
                                 name=f"rb_{nc.next_id()}")
                    MM(rsb, ones_row_r, rsr, start=True, stop=True)
                    mub_bf = P.tile([128, 512], BF, tag="scr", bufs=5,
                                    name=f"mbb_{nc.next_id()}")
                    nc.scalar.activation(out=mub_bf, in_=mub, func=AF.Copy)
                    rsb_bf = P.tile([128, 512], BF, tag="scr", bufs=5,
                                    name=f"rbb_{nc.next_id()}")
                    nc.scalar.activation(out=rsb_bf, in_=rsb, func=AF.Copy)
                    for kt in range(DT):
                        t1 = P.tile([128, 512], BF, tag="scr", bufs=5,
                                    name=f"t1_{nc.next_id()}")
                        nc.vector.tensor_tensor(t1, res[kt][:, sl], mub_bf,
                                                OP.subtract)
                        t2 = P.tile([128, 512], BF, tag="scr", bufs=5,
                                    name=f"t2_{nc.next_id()}")
                        nc.vector.scalar_tensor_tensor(
                            t2, t1, col(cs + kt), rsb_bf,
                            op0=OP.mult, op1=OP.mult)
                        nc.scalar.activation(out=dst[kt][:, sl], in_=t2,
                                             func=AF.Identity,
                                             bias=col(cb + kt))

            # ---------------- transformer layers ----------------
            for l in range(NLAYERS):
                AB = l * PL
                fw_sb = load_fw(l)       # lands during attention

                probe("xt", xT[0])
                qT, qT_tag = alloc_act()
                proj_v(qT, AW_Q, AB + 0, xT)
                probe("q", qT[0])
                kT, kT_tag = alloc_act()
                proj_v(kT, AW_K, AB + 4, xT)
                probe("k", kT[0])

                # V token-major (ones-row matmul adds bias)
                for rt in range(8):
                    ps = Q.tile([128, 512], F32, tag="C", bufs=2,
                                name=f"v_{nc.next_id()}")
                    for kt in range(DT):
                        MM(ps, xT[kt][:, rt * 128:(rt + 1) * 128],
                           aw_sb[:, AW_V + kt * 512:AW_V + kt * 512 + 512],
                           start=(kt == 0), stop=False)
                    MM(ps, ones_row_r, bvr_sb[l], start=False, stop=True)
                    v3o = vv[rt].rearrange("p (h e) -> p h e", e=DK + 1)
                    nc.vector.tensor_copy(
                        v3o[:, :, 0:DK],
                        ps.rearrange("p (h d) -> p h d", d=DK))

                probe("v", vv[0])
                gT, gT_tag = alloc_act()
                proj_v(gT, AW_G, AB + 12, xT, act=AF.Tanh)
                probe("g", gT[0])

                # ---------------- attention ----------------
                ctxT, ctx_tag = alloc_act()
                for b in range(BL):
                    for hp in range(4):
                        prt = [[None, None], [None, None]]
                        for h01 in range(2):
                            hs = slice(h01 * 64, h01 * 64 + 64)
                            for jp in range(2):
                                psA = Q.tile([128, 1024], F32, tag="A",
                                             bufs=2,
                                             name=f"sc_{nc.next_id()}")
                                for j2 in range(2):
                                    jt = jp * 2 + j2
                                    MM(psA[:, j2 * 512:(j2 + 1) * 512],
                                       kT[hp][hs, b * 512 + jt * 128:
                                              b * 512 + jt * 128 + 128],
                                       qT[hp][hs, b * 512:(b + 1) * 512],
                                       start=True, stop=True)
                                pr = P.tile([128, 1024], BF, tag="pr",
                                            bufs=4,
                                            name=f"pr_{nc.next_id()}")
                                nc.scalar.activation(out=pr, in_=psA,
                                                     func=AF.Exp)
                                nc.vector.tensor_mul(
                                    pr, pr,
                                    posE_sb[:, jp * 1024:(jp + 1) * 1024])
                                probe("pr", pr)
                                prt[h01][jp] = pr
                        for h01 in range(2):
                            h = hp * 2 + h01
                            pc = Q.tile([128, 512], F32, tag="B", bufs=2,
                                        name=f"pc_{nc.next_id()}")
                            for jt in range(4):
                                MM(pc[0:DK + 1, :],
                                   vv[b * 4 + jt][:, h * (DK + 1):
                                                  (h + 1) * (DK + 1)],
                                   prt[h01][jt // 2][:, (jt % 2) * 512:
                                                     (jt % 2) * 512 + 512],
                                   start=(jt == 0), stop=(jt == 3))
                            probe("pc", pc[0:DK + 1, :])
                            dcp = P.tile([1, 512], F32, tag="rden", bufs=3,
                                         name=f"dc_{nc.next_id()}")
                            nc.vector.tensor_copy(dcp, pc[DK:DK + 1, :])
                            rden = P.tile([1, 512], F32, tag="rden", bufs=3,
                                          name=f"rd_{nc.next_id()}")
                            nc.vector.reciprocal_approx_fast(
                                out=rden, in_=dcp)
                            rdr = P.tile([1, 512], F32R, tag="rden", bufs=3,
                                         name=f"rdr_{nc.next_id()}")
                            nc.vector.tensor_copy(rdr, rden)
                            pbc = Q.tile([64, 512], F32, tag="C", bufs=2,
                                         name=f"bc_{nc.next_id()}")
                            MM(pbc, ones_row_r[:, 0:64], rdr,
                               start=True, stop=True)
                            ctmp = P.tile([64, 512], BF, tag="ctmp", bufs=3,
                                          name=f"ct_{nc.next_id()}")
                            nc.scalar.activation(out=ctmp, in_=pc[0:DK, :],
                                                 func=AF.Copy)
                            nc.vector.tensor_mul(
                                ctxT[hp][h01 * 64:h01 * 64 + 64,
                                         b * 512:(b + 1) * 512],
                                ctmp, pbc)
                free_act(qT_tag)
                free_act(kT_tag)

                probe("ctx", ctxT[0])
                attT, attT_tag = alloc_act()
                proj_v(attT, AW_O, AB + 8, ctxT)
                probe("att", attT[0])
                free_act(ctx_tag)

                # res = x + (tanh+1) * att'  (att' pre-halved via Wo')
                res, res_tag = alloc_act()
                for kt in range(DT):
                    for rc in range(2):
                        sl = slice(rc * 512, (rc + 1) * 512)
                        tm = P.tile([128, 512], BF, tag="scr", bufs=5,
                                    name=f"tm_{nc.next_id()}")
                        nc.vector.scalar_tensor_tensor(
                            tm, gT[kt][:, sl], 1.0, attT[kt][:, sl],
                            op0=OP.add, op1=OP.mult)
                        nc.vector.tensor_add(res[kt][:, sl], tm,
                                             xT[kt][:, sl])
                free_act(xT_tag)
                free_act(gT_tag)
                free_act(attT_tag)

                probe("res", res[0])
                x1, x1_tag = alloc_act()
                layernorm(res, AB + 16, AB + 20, x1)
                probe("x1", x1[0])
                free_act(res_tag)

                # prefetch next layer's attention weights
                if l + 1 < NLAYERS:
                    aw_next = load_aw(l + 1)

                # ---------------- FF ----------------
                res2, res2_tag = alloc_act()
                for rc in range(2):
                    sl = slice(rc * 512, (rc + 1) * 512)
                    accA = [Q.tile([128, 1024], F32, tag="A", bufs=2,
                                   name=f"fa_{nc.next_id()}")
                            for _ in range(2)]
                    accs = [accA[0][:, 0:512], accA[0][:, 512:1024],
                            accA[1][:, 0:512], accA[1][:, 512:1024]]
                    for nt in range(FT):
                        p1 = Q.tile([128, 512], F32, tag="B", bufs=2,
                                    name=f"p1_{nc.next_id()}")
                        for kt in range(DT):
                            MM(p1,
                               fw_sb[:, FW_1 + kt * 2048 + nt * 128:
                                     FW_1 + kt * 2048 + nt * 128 + 128],
                               x1[kt][:, sl],
                               start=(kt == 0), stop=(kt == DT - 1))
                        pg = Q.tile([128, 512], F32, tag="C", bufs=2,
                                    name=f"pg_{nc.next_id()}")
                        for kt in range(DT):
                            MM(pg,
                               fw_sb[:, FW_G + kt * 2048 + nt * 128:
                                     FW_G + kt * 2048 + nt * 128 + 128],
                               x1[kt][:, sl],
                               start=(kt == 0), stop=(kt == DT - 1))
                        a1 = P.tile([128, 512], BF, tag="fsc", bufs=5,
                                    name=f"a1_{nc.next_id()}")
                        nc.vector.tensor_scalar(
                            out=a1, in0=p1, scalar1=col(AB + 32 + nt),
                            scalar2=None, op0=OP.add)
                        tg = P.tile([128, 512], BF, tag="fsc", bufs=5,
                                    name=f"tg_{nc.next_id()}")
                        nc.scalar.activation(out=tg, in_=pg, func=AF.Tanh,
                                             bias=col(AB + 48 + nt))
                        f = P.tile([128, 512], BF, tag="fsc", bufs=5,
                                   name=f"f_{nc.next_id()}")
                        nc.vector.scalar_tensor_tensor(
                            f, tg, 1.0, a1, op0=OP.add, op1=OP.mult)
                        for dt_ in range(DT):
                            MM(accs[dt_],
                               fw_sb[:, FW_2 + nt * 512 + dt_ * 128:
                                     FW_2 + nt * 512 + dt_ * 128 + 128],
                               f, start=(nt == 0), stop=(nt == FT - 1))
                    for dt_ in range(DT):
                        nc.vector.scalar_tensor_tensor(
                            res2[dt_][:, sl], accs[dt_], col(AB + 64 + dt_),
                            x1[dt_][:, sl], op0=OP.add, op1=OP.add)
                probe("res2", res2[0])
                free_act(x1_tag)

                xT, xT_tag = alloc_act()
                layernorm(res2, AB + 24, AB + 28, xT)
                probe("xout", xT[0])
                free_act(res2_tag)
                if l + 1 < NLAYERS:
                    aw_sb = aw_next

            # ---------------- head ----------------
            hT = []
            for kt in range(DT):
                xr = P.tile([128, BL], F32, tag="hd", bufs=8,
                            name=f"xr_{nc.next_id()}")
                nc.vector.tensor_reduce(
                    xr, xT[kt].rearrange("p (b s) -> p b s", b=BL),
                    axis=mybir.AxisListType.X, op=OP.add)
                ht = P.tile([128, BL], BF, tag="hT", bufs=8,
                            name=f"hm_{nc.next_id()}")
                nc.vector.tensor_scalar(out=ht, in0=xr, scalar1=1.0 / S,
                                        scalar2=None, op0=OP.mult)
                hT.append(ht)
            for nt in range(DT):
                ps = Q.tile([128, BL], F32, tag="B", bufs=2,
                            name=f"ho_{nc.next_id()}")
                MM(ps, ow_sb[:, nt * 128:(nt + 1) * 128], xo_sb,
                   start=True, stop=True)
                ht = P.tile([128, BL], BF, tag="hT", bufs=8,
                            name=f"hx_{nc.next_id()}")
                nc.vector.tensor_scalar(out=ht, in0=ps,
                                        scalar1=col(HB + 4 + nt),
                                        scalar2=None, op0=OP.add)
                hT.append(ht)

            eps1 = eps2[0:1, :]

            def head_ln_relu(zt, n_tiles, nfeat, cs, cb, outtag):
                s1p = Q.tile([1, BL], F32, tag="B", bufs=2,
                             name=f"hs1_{nc.next_id()}")
                for kt in range(n_tiles):
                    MM(s1p, ones_col, zt[kt], start=(kt == 0),
                       stop=(kt == n_tiles - 1))
                s2p = Q.tile([1, BL], F32, tag="C", bufs=2,
                             name=f"hs2_{nc.next_id()}")
                for kt in range(n_tiles):
                    z2 = P.tile([128, BL], BF, tag="hd2", bufs=4,
                                name=f"z2_{nc.next_id()}")
                    nc.vector.tensor_mul(z2, zt[kt], zt[kt])
                    MM(s2p, ones_col, z2, start=(kt == 0),
                       stop=(kt == n_tiles - 1))
                mu = P.tile([1, BL], F32R, tag="hmu", bufs=4,
                            name=f"hmu_{nc.next_id()}")
                nc.vector.tensor_scalar(out=mu, in0=s1p,
                                        scalar1=1.0 / nfeat,
                                        scalar2=None, op0=OP.mult)
                m2 = P.tile([1, BL], F32, tag="hln", bufs=8,
                            name=f"hm2_{nc.next_id()}")
                nc.vector.tensor_scalar(out=m2, in0=s2p,
                                        scalar1=1.0 / nfeat,
                                        scalar2=None, op0=OP.mult)
                var = P.tile([1, BL], F32, tag="hln", bufs=8,
                             name=f"hva_{nc.next_id()}")
                nc.vector.scalar_tensor_tensor(
                    var, mu, -1.0, mu, op0=OP.mult, op1=OP.mult)
                nc.vector.tensor_add(var, var, m2)
                sq = P.tile([1, BL], F32, tag="hln", bufs=8,
                            name=f"hsq_{nc.next_id()}")
                nc.scalar.activation(out=sq, in_=var, func=AF.Sqrt,
                                     bias=eps1)
                rs = P.tile([1, BL], F32, tag="hmu", bufs=4,
                            name=f"hrs_{nc.next_id()}")
                nc.vector.reciprocal_approx_fast(out=rs, in_=sq)
                rsr = P.tile([1, BL], F32R, tag="hmu", bufs=4,
                             name=f"hrr_{nc.next_id()}")
                nc.vector.tensor_copy(rsr, rs)
                mub = Q.tile([128, BL], F32, tag="C", bufs=2,
                             name=f"hmb_{nc.next_id()}")
                MM(mub, ones_row_r, mu, start=True, stop=True)
                rsb = Q.tile([128, BL], F32, tag="B", bufs=2,
                             name=f"hrb_{nc.next_id()}")
                MM(rsb, ones_row_r, rsr, start=True, stop=True)
                outs = []
                for kt in range(n_tiles):
                    t1 = P.tile([128, BL], F32, tag="hd", bufs=8,
                                name=f"ht1_{nc.next_id()}")
                    nc.vector.tensor_tensor(t1, zt[kt], mub, OP.subtract)
                    t2 = P.tile([128, BL], F32, tag="hd", bufs=8,
                                name=f"ht2_{nc.next_id()}")
                    nc.vector.scalar_tensor_tensor(
                        t2, t1, col(cs + kt), rsb, op0=OP.mult, op1=OP.mult)
                    o = P.tile([128, BL], BF, tag=outtag, bufs=4,
                               name=f"ho_{nc.next_id()}")
                    nc.scalar.activation(out=o, in_=t2, func=AF.Relu,
                                         bias=col(cb + kt))
                    outs.append(o)
                return outs

            # fc1 [1024 -> 256]
            z1 = []
            for nt in range(2):
                ps = Q.tile([128, BL], F32, tag="B", bufs=2,
                            name=f"f1_{nc.next_id()}")
                for kt in range(8):
                    MM(ps, hw1_sb[:, kt * 256 + nt * 128:
                                  kt * 256 + nt * 128 + 128], hT[kt],
                       start=(kt == 0), stop=(kt == 7))
                z = P.tile([128, BL], BF, tag="z1", bufs=2,
                           name=f"z1_{nc.next_id()}")
                nc.vector.tensor_scalar(out=z, in0=ps,
                                        scalar1=col(HB + 8 + nt),
                                        scalar2=None, op0=OP.add)
                z1.append(z)
            h1 = head_ln_relu(z1, 2, 256, HB + 10, HB + 12, "h1")

            # fc2 [256 -> 128]
            ps = Q.tile([128, BL], F32, tag="B", bufs=2,
                        name=f"f2_{nc.next_id()}")
            for kt in range(2):
                MM(ps, hw2_sb[:, kt * 128:(kt + 1) * 128], h1[kt],
                   start=(kt == 0), stop=(kt == 1))
            z2_ = P.tile([128, BL], BF, tag="z2", bufs=2,
                         name=f"z2h_{nc.next_id()}")
            nc.vector.tensor_scalar(out=z2_, in0=ps, scalar1=col(HB + 14),
                                    scalar2=None, op0=OP.add)
            h2 = head_ln_relu([z2_], 1, 128, HB + 15, HB + 16, "h2")

            # fc3 [128 -> 1]
            ps = Q.tile([1, BL], F32, tag="C", bufs=2,
                        name=f"f3_{nc.next_id()}")
            MM(ps, hw2_sb[:, 256:257], h2[0], start=True, stop=True)
            out_sb = P.tile([1, BL], F32, tag="outsb", bufs=1)
            nc.vector.tensor_scalar(out=out_sb, in0=ps,
                                    scalar1=pb_sb[0:1, HB + 18:HB + 19],
                                    scalar2=None, op0=OP.add)
            nc.sync.dma_start(out=out_ext[:, :], in_=out_sb)

    nc.compile()
    return nc


def _tile_w(W):
    """[K*128, Dout] -> [128, K*Dout] bf16 (kt-major blocks)."""
    K = W.shape[0] // 128
    return np.ascontiguousarray(
        W.reshape(K, 128, -1).transpose(1, 0, 2).reshape(128, -1)
    ).astype(NPBF)


def _cols(pb, base, vec):
    """Pack vec[len=128*n] into pb[:, base:base+n] column-major."""
    v = np.asarray(vec, np.float32).reshape(-1, 128).T
    pb[:, base:base + v.shape[1]] = v


def _pack_shared(inputs):
    f32 = np.float32
    g = lambda k: np.asarray(inputs[k], f32)

    aw = np.zeros((L, 128, AW_COLS), NPBF)
    fw = np.zeros((L, 128, FW_COLS), NPBF)
    pb = np.zeros((128, NP), f32)
    bvr = np.zeros((L, HDK), f32)
    Wq, bq = g("Wq"), g("bq")
    Wk, bk = g("Wk"), g("bk")
    Wv, bv = g("Wv"), g("bv")
    Wo, bo = g("Wo"), g("bo")
    Wg, bg = g("Wg"), g("bg")
    Wf1, bf1 = g("Wf1"), g("bf1")
    Wfg, bfg = g("Wfg"), g("bfg")
    Wf2, bf2 = g("Wf2"), g("bf2")
    for l in range(L):
        aw[l][:, AW_Q:AW_K] = _tile_w(Wq[l] * 0.125)
        aw[l][:, AW_K:AW_V] = _tile_w(Wk[l])
        aw[l][:, AW_V:AW_O] = _tile_w(Wv[l])
        aw[l][:, AW_O:AW_G] = _tile_w(Wo[l] * 0.5)
        aw[l][:, AW_G:] = _tile_w(Wg[l] * 0.5)
        fw[l][:, FW_1:FW_G] = _tile_w(Wf1[l] * 0.5)
        fw[l][:, FW_G:FW_2] = _tile_w(Wfg[l] * 0.5)
        fw[l][:, FW_2:] = _tile_w(Wf2[l])
        AB = l * PL
        _cols(pb, AB + 0, bq[l] * 0.125)
        _cols(pb, AB + 4, bk[l])
        _cols(pb, AB + 8, bo[l] * 0.5)
        _cols(pb, AB + 12, bg[l] * 0.5)
        _cols(pb, AB + 16, g("ln1_s")[l])
        _cols(pb, AB + 20, g("ln1_b")[l])
        _cols(pb, AB + 24, g("ln2_s")[l])
        _cols(pb, AB + 28, g("ln2_b")[l])
        _cols(pb, AB + 32, bf1[l] * 0.5)
        _cols(pb, AB + 48, bfg[l] * 0.5)
        _cols(pb, AB + 64, bf2[l])
        bvr[l] = bv[l]
    _cols(pb, HB + 0, g("cgm_b"))
    _cols(pb, HB + 4, g("other_b"))
    _cols(pb, HB + 8, g("fb1"))
    _cols(pb, HB + 10, g("fln1_s"))
    _cols(pb, HB + 12, g("fln1_b"))
    pb[:, HB + 14] = g("fb2")
    pb[:, HB + 15] = g("fln2_s")
    pb[:, HB + 16] = g("fln2_b")
    pb[:, HB + 17] = g("fW3")[:, 0]
    pb[0, HB + 18] = g("fb3")[0]

    # posE: exp(pos_bias) in scores-transposed layout
    rbar = g("rel_emb").mean(axis=1)            # [1023]
    posE = np.zeros((128, 2048), f32)
    Jv = np.arange(128)[:, None]
    Iv = np.arange(512)[None, :]
    for jt in range(4):
        idx = 511 - 128 * jt - Jv + Iv
        posE[:, jt * 512:(jt + 1) * 512] = np.exp(rbar[idx])

    return {
        "cgmW": g("cgm_W").astype(NPBF),
        "posE": posE.astype(NPBF),
        "aw": aw, "fw": fw, "pb": pb, "bvr": bvr,
        "hw1": _tile_w(g("fW1")),
        "hw2": np.concatenate([_tile_w(g("fW2")),
                               g("fW3").astype(NPBF)], axis=1),
        "ow": g("other_W").astype(NPBF),
    }


def _get_nc():
    if "nc" not in _CACHE:
        _CACHE["nc"] = _build()
    return _CACHE["nc"]


def kernel(**inputs):
    shared = _pack_shared(inputs)
    x_cgm = np.asarray(inputs["x_cgm"], np.float32)
    x_other = np.asarray(inputs["x_other"], np.float32)
    in_maps = []
    for c in range(NCORES):
        m = dict(shared)
        xs = x_cgm[c * BL:(c + 1) * BL].reshape(R, FC).T
        m["xin"] = np.ascontiguousarray(xs).astype(NPBF)
        m["xo"] = np.ascontiguousarray(
            x_other[c * BL:(c + 1) * BL].T).astype(NPBF)
        in_maps.append(m)

    nc = _get_nc()
    trace = bool(int(os.environ.get("KTRACE", "0")))
    res = run_bass_kernel_spmd(nc, in_maps, core_ids=list(range(NCORES)),
                               trace=trace)
    _CACHE["last_res"] = res
    out = np.concatenate(
        [res.results[c]["out"].reshape(BL, 1) for c in range(NCORES)], axis=0)
    return out.astype(np.float32)
